# revision 38
# baseline (speedup 1.0000x reference)
"""Bass/Tile SPMD kernel for GQA attention prefill (B=2,S=2048,D=4096,H=32,KVH=8,HD=128).

Head-sharded layout: 8 cores = 2 batch-groups x 4 head-group cores.
Core c: batch b=c//4, head-group hg=c%4 owns q-heads [8hg, 8hg+8) and
kv-heads [2hg, 2hg+2), projecting them for ALL 2048 rows of its batch.
No K/V/Q collectives: every core computes exactly the Q/K/V it needs.

Per core:
  phase 1: K_T, V, Q_T projections. wk/wv/wq all resident in SBUF
          (loaded once); x streamed in 8 column-chunks of 256 so the PE
          starts after ~2.6MB of DMA and never waits on weight traffic.
          RoPE on K and Q.
  phase 2: exact-causal attention per head over 4 query-slices of 512
          (blocks 2i,2i+1). k-tiles 0..4i+1 run 512-wide; the two tail
          k-tiles (diagonal of block 2i+1) run 256-wide. exp on ScalarE
          over [128,1024] 2-bank PSUM pairs; 0/1 mask only on diagonal
          tiles; rowsum via ones-matmul; normalize via
          reciprocal_approx_fast + DVE mul. Software pipeline keeps two
          QK score tiles in flight (3 PSUM bufs) so the ~1us exp never
          stalls the PE.
  xfer:   attention outputs AllGather'd in the 4-core batch group in 5
          chunks (heads [0,1],[2,3],[4,5],[6],[7]) - the last two are
          single-head so the tail collective is small and early. Each
          core extracts its own 512 query rows via indirect DMA (row
          table is per-core host data, program stays SPMD).
  phase 3: out rows = attn_T @ wo, wo streamed in 8 column-chunks. Each
          chunk's 32-head accumulation is split: 24 heads from AG chunks
          0-2 first (A-part), the 8 kch3 heads deferred (B-part), giving
          ~47us of AG-independent PE runway that hides the final
          collectives. Output rows disjoint across cores -> no reduction.
"""
import numpy as np
import concourse.bass as bass
import concourse.mybir as mybir
import concourse.tile as tile
from concourse import bacc

F32 = mybir.dt.float32
BF16 = mybir.dt.bfloat16

B, S, D = 2, 2048, 4096
H, KVH, HD = 32, 8, 128
NDT = D // 128              # 32 contraction tiles
NQH, NKVH = 8, 2            # heads per core
NCH, CW = 4, 512            # x column chunks
SCALE = float(1.0 / np.sqrt(HD))

# 4-rank mesh AG (8-rank always picks the slower RDH path). The CC
# stream is nearly saturated by the 8 chunks (~260us); phase 3 is
# restructured to need the tail chunks only ~190us after phase 2 ends.
REPLICA_GROUPS = [[0, 1, 2, 3], [4, 5, 6, 7]]
NRANKS = 4
# AllGather per head: per-head triggers start the CC stream as early
# as possible (right after head 0) so it drains inside phase 2/3-A.
AG_PLAN = [(h, (h,)) for h in range(NQH)]


def build():
    nc = bacc.Bacc("TRN2", target_bir_lowering=False, debug=False, num_devices=8)

    xt = nc.declare_dram_parameter("xt", [D, S], BF16, isOutput=False)
    wqt = nc.declare_dram_parameter("wqt", [NQH, NDT, 128, 128], BF16, isOutput=False)
    wkt = nc.declare_dram_parameter("wkt", [NDT, 128, NKVH * 128], BF16, isOutput=False)
    wvt = nc.declare_dram_parameter("wvt", [NDT, 128, NKVH * 128], BF16, isOutput=False)
    wot = nc.declare_dram_parameter("wot", [H * HD, D], BF16, isOutput=False)
    cos2 = nc.declare_dram_parameter("cos2", [128, S], BF16, isOutput=False)
    sin2 = nc.declare_dram_parameter("sin2", [128, S], BF16, isOutput=False)
    mdiag = nc.declare_dram_parameter("mdiag", [128, 512], BF16, isOutput=False)
    qidx = nc.declare_dram_parameter("qidx", [128, 8], mybir.dt.int32, isOutput=False)
    out = nc.declare_dram_parameter("out", [512, D], F32, isOutput=True)

    with tile.TileContext(nc) as tc:
        _body(nc, tc, xt.ap(), wqt.ap(), wkt.ap(), wvt.ap(), wot.ap(),
              cos2.ap(), sin2.ap(), mdiag.ap(), qidx.ap(), out.ap())

    nc.compile()
    return nc


def _body(nc, tc, xt, wqt, wkt, wvt, wot, cos2, sin2, mdiag, qidx, out):
    from contextlib import ExitStack

    es = ExitStack()
    with es:
        const_pool = es.enter_context(tc.tile_pool(name="consts", bufs=1))
        dram = es.enter_context(tc.tile_pool(name="dram", bufs=1, space="DRAM"))
        qkv_pool = es.enter_context(tc.tile_pool(name="qkv", bufs=1))

        cos_sb = const_pool.tile([128, S], BF16)
        sin_sb = const_pool.tile([128, S], BF16)
        md_sb = const_pool.tile([128, 2, 256], BF16)
        ones_sb = const_pool.tile([128, 128], BF16)   # rowsum stationary;
        # all-ones 128-wide so the rowsum matmul lands the row-sum on EVERY
        # psum partition - normalization then needs no partition broadcast
        qidx_sb = const_pool.tile([128, 8], mybir.dt.int32)  # gather rows
        nc.scalar.dma_start(qidx_sb[:], qidx)
        nc.scalar.dma_start(cos_sb[:], cos2)
        nc.scalar.dma_start(sin_sb[:], sin2)
        nc.scalar.dma_start(md_sb[:], mdiag.rearrange("p (h c) -> p h c", c=256))
        nc.vector.memset(ones_sb[:], 1.0)

        q_sb = qkv_pool.tile([128, NQH, S], BF16)     # Q_T head-major
        k_sb = qkv_pool.tile([128, NKVH, S], BF16)    # K_T kv-head-major
        v_sb = qkv_pool.tile([128, S // 128, NKVH * 128], BF16)  # V natural
        # attn redistribution AllGather buffers (sized per chunk's heads)
        ag_in = [dram.tile([len(hs) * 4 * 128 * 512], BF16, name=f"agin{k}")
                 for k, hs in AG_PLAN]
        ag_out = [dram.tile([NRANKS, len(hs) * 4 * 128 * 512], BF16,
                            name=f"agout{k}")
                  for k, hs in AG_PLAN]
        # tiny warmup AllGather: absorbs the first-collective overhead
        # (~40-100us) during phase 1 so the real AG stream runs steady
        warm_in = dram.tile([4096], BF16, name="warm_in")
        warm_out = dram.tile([NRANKS, 4096], BF16, name="warm_out")
        nc.gpsimd.collective_compute(
            "AllGather", mybir.AluOpType.bypass,
            replica_groups=REPLICA_GROUPS,
            ins=[warm_in.opt()], outs=[warm_out.opt()])
        xtr = xt.rearrange("(dt p) s -> p dt s", p=128)

        # ========== phase 1: K/V/Q projections, one x stream ==========
        with tc.tile_pool(name="p1w", bufs=1) as wpool, \
             tc.tile_pool(name="p1wq", bufs=3) as wqpool, \
             tc.tile_pool(name="p1x", bufs=2) as xpool, \
             tc.tile_pool(name="p1rope", bufs=2) as rpool, \
             tc.tile_pool(name="p1ps", bufs=2, space="PSUM") as pspool, \
             tc.tile_pool(name="p1psv", bufs=2, space="PSUM") as psvpool:

            wk_sb = wpool.tile([128, NDT, NKVH * 128], BF16)
            wv_sb = wpool.tile([128, NDT, NKVH * 128], BF16)
            wktr = wkt.rearrange("dt p e -> p dt e")
            # x chunk 0 first on the sync queue: the first matmul chain
            # needs only x0 + wk cols 0:128; startup is barrier-bound
            # (~25us rendezvous) so this fully hides the first stripe
            x0 = xpool.tile([128, NDT, CW], BF16, tag="xchunk")
            nc.sync.dma_start(x0[:], xtr[:, :, 0:CW])
            nc.sync.dma_start(wk_sb[:, :, 0:128], wktr[:, :, 0:128])

            def rope(ps, dst, q0c):
                raw = rpool.tile([128, CW], BF16, tag="rope_raw")
                sw_t = rpool.tile([128, CW], BF16, tag="rope_sw")
                nc.vector.tensor_copy(raw[:], ps)
                nc.scalar.dma_start(sw_t[0:64, :], raw[64:128, :])
                nc.scalar.dma_start(sw_t[64:128, :], raw[0:64, :])
                nc.vector.tensor_mul(dst, ps, cos_sb[:, q0c:q0c + CW])
                nc.vector.tensor_mul(
                    sw_t[:], sw_t[:], sin_sb[:, q0c:q0c + CW])
                nc.vector.tensor_add(dst, dst, sw_t[:])

            for ci in range(NCH):
                q0c = ci * CW
                if ci == 0:
                    x = x0
                    nc.sync.dma_start(wk_sb[:, :, 128:256], wktr[:, :, 128:256])
                    nc.sync.dma_start(
                        wv_sb[:], wvt.rearrange("dt p e -> p dt e"))
                else:
                    x = xpool.tile([128, NDT, CW], BF16, tag="xchunk")
                    nc.sync.dma_start(x[:], xtr[:, :, q0c:q0c + CW])
                for kv in range(NKVH):
                    kps = pspool.tile([128, CW], F32, tag="kps")
                    for dt in range(NDT):
                        nc.tensor.matmul(
                            kps[:], wk_sb[:, dt, kv * 128:(kv + 1) * 128],
                            x[:, dt], start=(dt == 0), stop=(dt == NDT - 1))
                    rope(kps[:], k_sb[:, kv, q0c:q0c + CW], q0c)
                vps = psvpool.tile([128, 4, 256], F32, tag="vps")
                # st sequential: one live accumulation group per PSUM bank
                # at a time (a second start=True in the same bank clears the
                # bank's has_written bits and drops the first group's sum)
                for st in range(CW // 128):
                    for dt in range(NDT):
                        nc.tensor.matmul(
                            vps[:, st], x[:, dt, st * 128:(st + 1) * 128],
                            wv_sb[:, dt], start=(dt == 0), stop=(dt == NDT - 1))
                    nc.vector.tensor_copy(
                        v_sb[:, q0c // 128 + st, :], vps[:, st])
                for et in range(NQH):
                    wq_c = wqpool.tile([128, NDT, 128], BF16, tag="wq")
                    nc.sync.dma_start(
                        wq_c[:], wqt[et].rearrange("dt p e -> p dt e"))
                    qps = pspool.tile([128, CW], F32, tag="qps")
                    for dt in range(NDT):
                        nc.tensor.matmul(
                            qps[:], wq_c[:, dt], x[:, dt],
                            start=(dt == 0), stop=(dt == NDT - 1))
                    rope(qps[:], q_sb[:, et, q0c:q0c + CW], q0c)

        # opened after the projection passes so their SBUF peaks don't stack
        at_pool = es.enter_context(tc.tile_pool(name="at", bufs=1))
        at_sb = at_pool.tile([128, H, 512], BF16)     # all 32 heads, my rows
        # wo split in three head-groups by AG arrival time
        GROUPS = [(0, 3), (3, 3), (6, 2)]   # (first head, n heads)
        wpools = [es.enter_context(tc.tile_pool(name=f"p3w{g}", bufs=2))
                  for g in range(3)]
        wotr = wot.rearrange("(et p) d -> p et d", p=128)
        wo8 = wotr.rearrange("p (s h8) d -> p s h8 d", h8=8)

        def load_wg(wG, gi, dch):
            h0, nh = GROUPS[gi]
            for s in range(4):
                nc.sync.dma_start(
                    wG[:, s], wo8[:, s, h0:h0 + nh, dch * 512:(dch + 1) * 512])

        wG00 = wpools[0].tile([128, 4, 3, 512], BF16, tag="wo0")
        load_wg(wG00, 0, 0)

        # ================= phase 2: exact-causal attention =================
        with tc.tile_pool(name="attn", bufs=1) as attn_pool, \
             tc.tile_pool(name="p2p", bufs=2) as ppool, \
             tc.tile_pool(name="p2n", bufs=2) as npool, \
             tc.tile_pool(name="p2ps_s", bufs=3, space="PSUM") as ps_s, \
             tc.tile_pool(name="p2ps_o", bufs=1, space="PSUM") as ps_o, \
             tc.tile_pool(name="p2ps_r", bufs=1, space="PSUM") as ps_r:

            attn_sb = attn_pool.tile([128, 2, S], BF16)  # head-pair ring
            for h in range(NQH):
                kv = h // 4
                for i in range(4):          # query slice: blocks 2i, 2i+1
                    q0 = i * 512
                    npair = 2 * i + 1       # 512-wide kt pairs
                    # p_big[p, t, half, q]: exp'd probs, key-tile-pair major
                    p_big = ppool.tile([128, 8, 2, 512], BF16, tag="p_big")
                    ops = ps_o.tile([128, 512], F32, tag="ops")
                    rps = ps_r.tile([128, 512], F32, tag="rps")

                    def qk_pair(t):
                        sps = ps_s.tile([128, 2, 512], F32, tag="sps")
                        nc.tensor.matmul(
                            sps[:, 0], k_sb[:, kv, (2 * t) * 128:(2 * t + 1) * 128],
                            q_sb[:, h, q0:q0 + 512], start=True, stop=True)
                        nc.tensor.matmul(
                            sps[:, 1], k_sb[:, kv, (2 * t + 1) * 128:(2 * t + 2) * 128],
                            q_sb[:, h, q0:q0 + 512], start=True, stop=True)
                        nc.scalar.activation(
                            p_big[:, t], sps[:],
                            mybir.ActivationFunctionType.Exp)
                        if t == npair - 1:
                            # diagonal of block 2i: cols 0:256 of both halves
                            nc.vector.tensor_mul(
                                p_big[:, t, :, 0:256], p_big[:, t, :, 0:256],
                                md_sb[:])

                    def qk_tail():
                        # kt=4i+2, 4i+3: diagonal of block 2i+1 (cols 256:512)
                        tps = ps_s.tile([128, 2, 512], F32, tag="sps")
                        nc.tensor.matmul(
                            tps[:, 0, 0:256],
                            k_sb[:, kv, (4 * i + 2) * 128:(4 * i + 3) * 128],
                            q_sb[:, h, q0 + 256:q0 + 512], start=True, stop=True)
                        nc.tensor.matmul(
                            tps[:, 1, 0:256],
                            k_sb[:, kv, (4 * i + 3) * 128:(4 * i + 4) * 128],
                            q_sb[:, h, q0 + 256:q0 + 512], start=True, stop=True)
                        nc.scalar.activation(
                            p_big[:, npair, :, 0:256], tps[:, :, 0:256],
                            mybir.ActivationFunctionType.Exp)
                        nc.vector.tensor_mul(
                            p_big[:, npair, :, 0:256], p_big[:, npair, :, 0:256],
                            md_sb[:])

                    def pv_pair(t):
                        for half in range(2):
                            nc.tensor.matmul(
                                ops[:],
                                v_sb[:, 2 * t + half, kv * 128:(kv + 1) * 128],
                                p_big[:, t, half], start=(t == 0 and half == 0),
                                stop=False, skip_group_check=True)
                        # pre-sum the two halves off the PE (fp32, exact)
                        # so the ones-matmul streams half the columns
                        psm = npool.tile([128, 512], BF16, tag="psum_pair")
                        nc.vector.tensor_add(
                            psm[:], p_big[:, t, 0], p_big[:, t, 1])
                        nc.tensor.matmul(
                            rps[:], ones_sb[:], psm[:],
                            start=(t == 0), stop=False,
                            skip_group_check=True)

                    def pv_tail():
                        for half in range(2):
                            nc.tensor.matmul(
                                ops[:, 256:512],
                                v_sb[:, 4 * i + 2 + half, kv * 128:(kv + 1) * 128],
                                p_big[:, npair, half, 0:256],
                                start=False, stop=(half == 1),
                                skip_group_check=True)
                        psm = npool.tile([128, 512], BF16, tag="psum_pair")
                        nc.vector.tensor_add(
                            psm[:, 0:256], p_big[:, npair, 0, 0:256],
                            p_big[:, npair, 1, 0:256])
                        nc.tensor.matmul(
                            rps[:, 256:512], ones_sb[:], psm[:, 0:256],
                            start=False, stop=True,
                            skip_group_check=True)

                    # software pipeline, lookahead 2: QK(t+1) and QK(t+2)
                    # are in flight before PV(t), so the ~1us exp of
                    # segment t is fully covered by PE work (3 sps bufs)
                    nissued = 0

                    def issue_qk():
                        nonlocal nissued
                        if nissued < npair:
                            qk_pair(nissued)
                        elif nissued == npair:
                            qk_tail()
                        nissued += 1

                    for _ in range(min(3, npair + 1)):
                        issue_qk()
                    for t in range(npair):
                        pv_pair(t)
                        issue_qk()
                    pv_tail()
                    # --- normalize ---
                    rcp = npool.tile([128, 512], F32, tag="rcp")
                    nc.vector.reciprocal_approx_fast(rcp[:], rps[:])
                    nc.vector.tensor_mul(
                        attn_sb[:, h % 2, q0:q0 + 512], ops[:], rcp[:])

                def gather_chunk(kch, hs):
                    # extract my 512 query rows from the gathered copies;
                    # row index src*512 + hg*128 + p (within my batch
                    # group's blocks) comes from per-core host data, so
                    # the program stays SPMD
                    nhp = len(hs)
                    agr = ag_out[kch].rearrange(
                        "src (hp r c) -> (src hp r) c", hp=nhp, c=512)
                    for src in range(4):
                        for hp_i, h_ in enumerate(hs):
                            et = src * NQH + h_
                            j = src * nhp + hp_i
                            nc.gpsimd.indirect_dma_start(
                                out=at_sb[:, et, :], out_offset=None,
                                in_=agr,
                                in_offset=bass.IndirectOffsetOnAxis(
                                    ap=qidx_sb[:, j:j + 1], axis=0))

                for kch, hs in AG_PLAN:
                    if hs[-1] != h:
                        continue
                    # AllGather this chunk's heads
                    nhp = len(hs)
                    agw = ag_in[kch].rearrange(
                        "(hp qb p c) -> hp qb p c", hp=nhp, qb=4, p=128)
                    for hp_i, h_ in enumerate(hs):
                        for qb in range(4):
                            nc.sync.dma_start(
                                agw[hp_i, qb],
                                attn_sb[:, h_ % 2, qb * 512:(qb + 1) * 512])
                    nc.gpsimd.collective_compute(
                        "AllGather", mybir.AluOpType.bypass,
                        replica_groups=REPLICA_GROUPS,
                        ins=[ag_in[kch].opt()], outs=[ag_out[kch].opt()])
                    # gathers run one chunk behind the triggers: chunk
                    # kch-1's AG is long done, so its gathers never make
                    # the NEXT trigger wait in the GpSimd FIFO
                    if kch > 0:
                        gather_chunk(*AG_PLAN[kch - 1])
                    if kch == len(AG_PLAN) - 1:
                        gather_chunk(kch, hs)

        # ====== phase 3: output projection (my 512 rows, all 32 heads) =====
        # Three passes over all 8 wo-chunks, one per head-group, ordered
        # by AG arrival: group 0 (heads 0-2) needs only the first AGs;
        # group 1 (3-5) isn't reached until ~100us into phase 3; group 2
        # (6,7) ~200us in - robust to a slow/late collective stream.
        # Partials park in DRAM as bf16 between passes; the running sum
        # is restored during each pass's evacuation on DVE.
        part_dram = dram.tile([8, 4, 128, 512], BF16, name="part")
        with tc.tile_pool(name="p3y", bufs=4) as ypool, \
             tc.tile_pool(name="p3stg", bufs=12) as stgpool, \
             tc.tile_pool(name="p3ps", bufs=4, space="PSUM") as ps_y:

            for gi, (h0, nh) in enumerate(GROUPS):
                for dch in range(8):
                    if gi == 0 and dch == 0:
                        wG = wG00
                    else:
                        wG = wpools[gi].tile(
                            [128, 4, nh, 512], BF16, tag=f"wo{gi}")
                        load_wg(wG, gi, dch)
                    stgs = None
                    if gi > 0:
                        stgs = []
                        for st in range(4):
                            stg = stgpool.tile([128, 512], BF16, tag="pstage")
                            nc.scalar.dma_start(stg[:], part_dram[dch, st])
                            stgs.append(stg)
                    ypsA = ps_y.tile([128, 2, 512], F32, tag="yps",
                                     name="ypsA")
                    ypsB = ps_y.tile([128, 2, 512], F32, tag="yps",
                                     name="ypsB")

                    def ysl(st):
                        return ypsA[:, st] if st < 2 else ypsB[:, st - 2]

                    last = nh * 4 - 1
                    for j, (hh, s) in enumerate(
                            (h0 + k, s) for k in range(nh) for s in range(4)):
                        et = s * NQH + hh
                        for st in range(4):
                            nc.tensor.matmul(
                                ysl(st),
                                at_sb[:, et, st * 128:(st + 1) * 128],
                                wG[:, s, hh - h0], start=(j == 0),
                                stop=(j == last), skip_group_check=True)
                    for st in range(4):
                        if gi < 2:
                            stg = stgpool.tile([128, 512], BF16, tag="pstage")
                            if gi == 0:
                                if st < 2:
                                    nc.scalar.copy(stg[:], ysl(st))
                                else:
                                    nc.vector.tensor_copy(stg[:], ysl(st))
                            else:
                                nc.vector.tensor_add(
                                    stg[:], ysl(st), stgs[st][:])
                            nc.scalar.dma_start(part_dram[dch, st], stg[:])
                        else:
                            y = ypool.tile([128, 512], F32, tag="y_sb")
                            nc.vector.tensor_add(y[:], ysl(st), stgs[st][:])
                            nc.sync.dma_start(
                                out[st * 128:(st + 1) * 128,
                                    dch * 512:(dch + 1) * 512], y[:])


# ======================= host side =======================

def _perm_idx(nheads):
    """Within each 128-dim head block: evens then odds."""
    idx = []
    for hh in range(nheads):
        base = hh * HD
        idx.extend(base + j for j in range(0, HD, 2))
        idx.extend(base + j for j in range(1, HD, 2))
    return np.array(idx)


def host_prep(x_norm, wq, wk, wv, wo, freqs_cos, freqs_sin, mask):
    """Build the 8 per-core input maps."""
    import ml_dtypes
    bf16 = ml_dtypes.bfloat16
    f32 = np.float32
    x_norm = np.ascontiguousarray(x_norm, dtype=f32)
    wq = np.asarray(wq, dtype=f32) * SCALE   # fold 1/sqrt(HD) into wq
    wk = np.asarray(wk, dtype=f32)
    wv = np.asarray(wv, dtype=f32)
    wo = np.asarray(wo, dtype=f32)
    fc = np.asarray(freqs_cos, dtype=f32)
    fs = np.asarray(freqs_sin, dtype=f32)

    pq = _perm_idx(H)
    pk = _perm_idx(KVH)
    wq_p = wq[pq, :]                     # [H*HD, D] permuted rows
    wk_p = wk[pk, :]

    cosT = fc.T                          # [64, S]
    sinT = fs.T
    cos_full = np.concatenate([cosT, cosT], axis=0).astype(bf16)   # [128, S]
    sin_full = np.concatenate([-sinT, sinT], axis=0).astype(bf16)

    # diagonal 0/1 mask, block-invariant: [k_rel 128, half, q 256]
    q_r = np.arange(256)
    k_r = np.arange(128)
    md = np.zeros((128, 2, 256), dtype=f32)
    md[:, 0, :] = (q_r[None, :] >= k_r[:, None])
    md[:, 1, :] = (q_r[None, :] >= (128 + k_r)[:, None])
    md = np.ascontiguousarray(md.reshape(128, 512)).astype(bf16)

    wot = np.ascontiguousarray(wo.T).astype(bf16)    # [H*HD, D]

    xt_b = [np.ascontiguousarray(x_norm[b].T).astype(bf16) for b in range(B)]

    in_maps = []
    for c in range(8):
        b, hg = c // 4, c % 4
        # wq slice: heads [8hg, 8hg+8) -> [NQH, NDT, 128, 128]
        wq_c = wq_p[hg * NQH * HD:(hg + 1) * NQH * HD, :]   # [1024, 4096]
        wq_t = wq_c.T.reshape(NDT, 128, NQH, 128)            # [dt, p, et, e]
        wqt = np.ascontiguousarray(wq_t.transpose(2, 0, 1, 3)).astype(bf16)
        # wk/wv slice: kv-heads [2hg, 2hg+2) -> [NDT, 128, 256]
        wk_c = wk_p[hg * NKVH * HD:(hg + 1) * NKVH * HD, :]
        wkt = np.ascontiguousarray(
            wk_c.T.reshape(NDT, 128, NKVH * 128)).astype(bf16)
        wv_c = wv[hg * NKVH * HD:(hg + 1) * NKVH * HD, :]
        wvt = np.ascontiguousarray(
            wv_c.T.reshape(NDT, 128, NKVH * 128)).astype(bf16)
        # per-head chunks: col j=src selects rank src's block; 4..7 unused
        qi = (np.arange(8)[None, :] * 512 + hg * 128
              + np.arange(128)[:, None]).astype(np.int32)

        in_maps.append({
            "xt": xt_b[b],
            "wqt": wqt, "wkt": wkt, "wvt": wvt, "wot": wot,
            "cos2": cos_full, "sin2": sin_full, "mdiag": md,
            "qidx": qi,
        })
    return in_maps


def assemble(results):
    """results: list of 8 dicts with 'out' [512, 4096] -> full [B, S, D]."""
    full = np.empty((B, S, D), dtype=np.float32)
    for c in range(8):
        b, hg = c // 4, c % 4
        full[b, hg * 512:(hg + 1) * 512] = results[c]["out"]
    return full


# ======================= public entry point =======================

_NC_CACHE = {}
last_exec_time_ns = None


def _get_nc():
    if "nc" not in _NC_CACHE:
        _NC_CACHE["nc"] = build()
    return _NC_CACHE["nc"]


def kernel(x_norm, wq, wk, wv, wo, freqs_cos, freqs_sin, mask, start_pos=0, **_):
    """GQA attention prefill on 8 TRN2 NeuronCores. Full inputs in, full output out."""
    import os
    global last_exec_time_ns
    from concourse.bass_utils import run_bass_kernel_spmd

    nc = _get_nc()
    in_maps = host_prep(x_norm, wq, wk, wv, wo, freqs_cos, freqs_sin, mask)
    trace = os.environ.get("BASS_KERNEL_TRACE", "0") == "1"
    res = run_bass_kernel_spmd(nc, in_maps, core_ids=list(range(8)), trace=trace)
    last_exec_time_ns = res.exec_time_ns
    _NC_CACHE["res"] = res
    return assemble(res.results)
